# revision 1
# baseline (speedup 1.0000x reference)
"""Bass/Tile kernel builder for nn_DirectionDPINN: RK4 scan + 6D->quat.

Layout: batch-on-partitions (128 batches/core), N chunked by F along free dim.
Scan reduces to two tensor_tensor_scan recurrences per component.
Quat: Gram-Schmidt in unnormalized form + symmetric-matrix column select via
copy_predicated, final normalization.
"""
import math
import numpy as np
import concourse.bass as bass
import concourse.bacc as bacc
import concourse.mybir as mybir
from concourse.mybir import AluOpType as Op, ActivationFunctionType as AF
from concourse.tile import TileContext

F32 = mybir.dt.float32
P = 128
EPS = 1e-8

# 2-input op sites assigned to Pool (GpSimd); everything else 2-input -> DVE.
POOL_SITES_DEFAULT = frozenset({
    # strided sites (Pool's relative strided penalty is lower than DVE's)
    "s0", "s1", "s2",
    "w0", "w1", "w2",
    "cm0", "cm1", "cm2",
    "r2om0", "r2om1", "r2om2",
    "r2os0", "r2os1", "r2os2",
    "b10", "b11", "b12",
    "crm0", "crm1", "crm2", "crm3", "crm4", "crm5",
    "fin0", "fin1",
    # contiguous overflow, sized from measured rates (DVE .674 vs Pool 1.214)
    "e0", "e1", "e2",
    "b20", "b21", "b22",
    "b30", "b31", "b32",
    "dp0",
})


def build_nc(BC=128, N=4096, F=512, pool_sites=POOL_SITES_DEFAULT,
             scan_engine="vector", detect_races=True, reps=1,
             skip_compute=False, skip_dma=False, ns=26, wk_bufs=1, io_bufs=2,
             no_scan=False, no_pred=False, no_quat=False):
    assert BC == P and N % F == 0
    NCH = N // F
    nc = bacc.Bacc("TRN2", target_bir_lowering=False, detect_race_conditions=detect_races)
    sixd = nc.dram_tensor("sixd", [BC, N, 6], F32, kind="ExternalInput")
    accel = nc.dram_tensor("accel", [BC, N, 3], F32, kind="ExternalInput")
    v0d = nc.dram_tensor("v0", [BC, 3], F32, kind="ExternalInput")
    z0d = nc.dram_tensor("z0", [BC, 3], F32, kind="ExternalInput")
    td = nc.dram_tensor("t", [BC, N, 1], F32, kind="ExternalInput")
    outd = nc.dram_tensor("out", [BC, N, 10], F32, kind="ExternalOutput")

    V, G, S = nc.vector, nc.gpsimd, nc.scalar

    def TT(site, out, a, b, op):
        eng = G if site in pool_sites else V
        return eng.tensor_tensor(out, a, b, op)

    def STT(site, out, in0, imm, in1, op0, op1):
        eng = G if site in pool_sites else V
        return eng.scalar_tensor_tensor(out, in0, float(imm), in1, op0, op1)

    SCAN_ENG = {"vector": V, "gpsimd": G}[scan_engine]

    with TileContext(nc) as tc:
        with tc.tile_pool(name="cst", bufs=1) as cst, \
             tc.tile_pool(name="io", bufs=io_bufs) as io, \
             tc.tile_pool(name="wk", bufs=wk_bufs) as wk:
            v0t = cst.tile([P, 3], F32, name="v0t")
            z0t = cst.tile([P, 3], F32, name="z0t")
            nc.sync.dma_start(v0t[:], v0d[:])
            nc.sync.dma_start(z0t[:], z0d[:])

            # persistent scan-section planes (by name, reused per chunk)
            from contextlib import nullcontext
            loop_ctx = tc.For_i(0, reps, 1) if reps > 1 else nullcontext()
            with loop_ctx:
              prev_out = None
              for ci in range(NCH):
                  n0 = ci * F
                  lo = 1 if ci == 0 else 0
                  sl = slice(lo, F)

                  sixt = io.tile([P, F, 6], F32, name="sixt")
                  att = io.tile([P, F + 1, 3], F32, name="att")
                  ttt = io.tile([P, F + 1], F32, name="ttt")
                  out_t = io.tile([P, F, 10], F32, name="out_t")
                  if not skip_dma:
                      if ci == 0:
                          nc.sync.dma_start(att[:, 1:, :], accel[:, 0:F, :])
                          nc.sync.dma_start(ttt[:, 1:], td[:, 0:F, 0])
                      else:
                          nc.sync.dma_start(att[:], accel[:, n0 - 1:n0 + F, :])
                          nc.sync.dma_start(ttt[:], td[:, n0 - 1:n0 + F, 0])
                      nc.sync.dma_start(sixt[:], sixd[:, n0:n0 + F, :])

                  if skip_compute:
                      prev_out = out_t
                      continue
                  # ---------------- scan section ----------------
                  dt = wk.tile([P, F], F32, name="dt")
                  dts = wk.tile([P, F], F32, name="dts")
                  TT("dt", dt[:, sl], ttt[:, lo + 1:F + 1], ttt[:, lo:F], Op.subtract)
                  dth = wk.tile([P, F], F32, name="dth")
                  V.tensor_scalar(dth[:, sl], dt[:, sl], 0.5, None, Op.mult)
                  # dts = dt^2/6
                  S.activation(dts[:, sl], dt[:, sl], AF.Square,
                               scale=float(math.sqrt(1.0 / 6.0)))
                  scan_tiles = []
                  for c in range(3):
                      av = att[:, :, c]
                      s_ = wk.tile([P, F], F32, name=f"s{c}")
                      g_ = wk.tile([P, F], F32, name=f"g{c}")
                      w_ = wk.tile([P, F], F32, name=f"w{c}")
                      e_ = wk.tile([P, F], F32, name=f"e{c}")
                      hv = wk.tile([P, F], F32, name=f"hv{c}")
                      TT(f"s{c}", s_[:, sl], av[:, lo:F], av[:, lo + 1:F + 1], Op.add)
                      TT(f"g{c}", g_[:, sl], s_[:, sl], dth[:, sl], Op.mult)
                      TT(f"w{c}", w_[:, sl], s_[:, sl], av[:, lo:F], Op.add)
                      TT(f"e{c}", e_[:, sl], w_[:, sl], dts[:, sl], Op.mult)
                      scan_tiles.append((g_, e_, hv))

                  # ---------------- quat section ----------------
                  NS = ns
                  slots = [wk.tile([P, F], F32, name=f"q{i}") for i in range(NS)]
                  free = list(range(NS))
                  def tmp():
                      # FIFO rotation: reuse the oldest-freed slot to maximize
                      # WAR slack (LIFO would chain each new write onto the
                      # just-freed slot and serialize the whole section).
                      return slots[free.pop(0)]
                  def rel(*aps):
                      for ap in aps:
                          free.append(slots.index(ap))
                  Xv = sixt[:, :, 0]
                  Yv = sixt[:, :, 1]
                  Zv = sixt[:, :, 2]
                  Pv = sixt[:, :, 3]
                  Qv = sixt[:, :, 4]
                  Rv = sixt[:, :, 5]
                  A_ = lambda t_: t_[:, 0:F]

                  # n1 = X^2+Y^2+Z^2
                  t1, t2, t3 = tmp(), tmp(), tmp()
                  S.activation(A_(t1), Xv, AF.Square)
                  S.activation(A_(t2), Yv, AF.Square)
                  TT("n1a0", A_(t3), A_(t1), A_(t2), Op.add)
                  S.activation(A_(t1), Zv, AF.Square)
                  n1 = t2
                  TT("n1a1", A_(n1), A_(t3), A_(t1), Op.add)
                  rel(t1, t3)
                  # c = X*P + Y*Q + Z*R
                  t1, t3, t4 = tmp(), tmp(), tmp()
                  TT("cm0", A_(t1), Xv, Pv, Op.mult)
                  TT("cm1", A_(t3), Yv, Qv, Op.mult)
                  TT("ca0", A_(t4), A_(t1), A_(t3), Op.add)
                  TT("cm2", A_(t1), Zv, Rv, Op.mult)
                  cdot = t3
                  TT("ca1", A_(cdot), A_(t4), A_(t1), Op.add)
                  rel(t1, t4)
                  # inv1 = 1/(sqrt(n1)+eps)
                  t1, t4 = tmp(), tmp()
                  S.activation(A_(t1), A_(n1), AF.Ln)
                  inv1 = t4
                  S.activation(A_(inv1), A_(t1), AF.Exp, scale=-0.5)
                  inv1sq = tmp()
                  S.activation(A_(inv1sq), A_(t1), AF.Exp, scale=-1.0)
                  rel(t1, n1)
                  k = tmp()
                  TT("k", A_(k), A_(cdot), A_(inv1sq), Op.mult)
                  rel(cdot, inv1sq)
                  # r2o = r2 - k*r1
                  t2b = tmp()
                  ox, oy, oz = tmp(), tmp(), tmp()
                  TT("r2om0", A_(t2b), A_(k), Xv, Op.mult)
                  TT("r2os0", A_(ox), Pv, A_(t2b), Op.subtract)
                  TT("r2om1", A_(t2b), A_(k), Yv, Op.mult)
                  TT("r2os1", A_(oy), Qv, A_(t2b), Op.subtract)
                  TT("r2om2", A_(t2b), A_(k), Zv, Op.mult)
                  TT("r2os2", A_(oz), Rv, A_(t2b), Op.subtract)
                  rel(k)
                  # n2
                  t4 = tmp()
                  S.activation(A_(t2b), A_(ox), AF.Square)
                  S.activation(A_(t4), A_(oy), AF.Square)
                  t5 = tmp()
                  TT("n2a0", A_(t5), A_(t2b), A_(t4), Op.add)
                  S.activation(A_(t2b), A_(oz), AF.Square)
                  n2 = t4
                  TT("n2a1", A_(n2), A_(t5), A_(t2b), Op.add)
                  rel(t5)
                  # inv2
                  S.activation(A_(t2b), A_(n2), AF.Ln)
                  inv2 = n2
                  S.activation(A_(inv2), A_(t2b), AF.Exp, scale=-0.5)
                  rel(t2b)
                  # b1 = r1*inv1
                  b1x, b1y, b1z = tmp(), tmp(), tmp()
                  TT("b10", A_(b1x), Xv, A_(inv1), Op.mult)
                  TT("b11", A_(b1y), Yv, A_(inv1), Op.mult)
                  TT("b12", A_(b1z), Zv, A_(inv1), Op.mult)
                  # b2 = r2o*inv2
                  b2x, b2y, b2z = tmp(), tmp(), tmp()
                  TT("b20", A_(b2x), A_(ox), A_(inv2), Op.mult)
                  TT("b21", A_(b2y), A_(oy), A_(inv2), Op.mult)
                  TT("b22", A_(b2z), A_(oz), A_(inv2), Op.mult)
                  rel(ox, oy, oz)
                  # s3 = inv1*inv2
                  s3 = tmp()
                  TT("s3", A_(s3), A_(inv1), A_(inv2), Op.mult)
                  rel(inv1, inv2)
                  # cross = r1 x r2
                  ta, tb = tmp(), tmp()
                  cx, cy, cz = tmp(), tmp(), tmp()
                  TT("crm0", A_(ta), Yv, Rv, Op.mult)
                  TT("crm1", A_(tb), Zv, Qv, Op.mult)
                  TT("crs0", A_(cx), A_(ta), A_(tb), Op.subtract)
                  TT("crm2", A_(ta), Zv, Pv, Op.mult)
                  TT("crm3", A_(tb), Xv, Rv, Op.mult)
                  TT("crs1", A_(cy), A_(ta), A_(tb), Op.subtract)
                  TT("crm4", A_(ta), Xv, Qv, Op.mult)
                  TT("crm5", A_(tb), Yv, Pv, Op.mult)
                  TT("crs2", A_(cz), A_(ta), A_(tb), Op.subtract)
                  # b3 = cross*s3
                  b3x = ta
                  b3y = tb
                  TT("b30", A_(b3x), A_(cx), A_(s3), Op.mult)
                  TT("b31", A_(b3y), A_(cy), A_(s3), Op.mult)
                  b3z = cx
                  TT("b32", A_(b3z), A_(cz), A_(s3), Op.mult)
                  rel(cy, cz, s3)
                  # masks (raw comparisons; uint8 for copy_predicated)
                  m2 = wk.tile([P, F], mybir.dt.uint8, name="m2u")
                  m3 = wk.tile([P, F], mybir.dt.uint8, name="m3u")
                  m1 = wk.tile([P, F], mybir.dt.uint8, name="m1u")
                  mxp = tmp()
                  TT("mx", A_(mxp), A_(b2y), A_(b3z), Op.max)
                  TT("m2", m2[:, 0:F], A_(b1x), A_(mxp), Op.is_gt)
                  TT("m3", m3[:, 0:F], A_(b2y), A_(b3z), Op.is_gt)
                  rel(mxp)
                  # diag
                  Ad, Bd = tmp(), tmp()
                  TT("Ad", A_(Ad), A_(b1x), A_(b2y), Op.add)
                  TT("Bd", A_(Bd), A_(b1x), A_(b2y), Op.subtract)
                  rel(b1x, b2y)
                  tr1pre, m00pre = tmp(), tmp()
                  A1, B1 = tmp(), tmp()
                  V.tensor_scalar(A_(A1), A_(Ad), 1.0, None, Op.add)
                  V.tensor_scalar(A_(B1), A_(Bd), 1.0, None, Op.add)
                  TT("tr1p", A_(tr1pre), A_(A1), A_(b3z), Op.add)
                  TT("m00p", A_(m00pre), A_(B1), A_(b3z), Op.subtract)
                  m11pre, m22pre = tmp(), tmp()
                  An, Bn = A1, B1
                  V.tensor_scalar(A_(Bn), A_(Bd), -2.0, None, Op.mult)
                  V.tensor_scalar(A_(An), A_(Ad), -2.0, None, Op.mult)
                  TT("m11p", A_(m11pre), A_(Bn), A_(m00pre), Op.add)
                  TT("m22p", A_(m22pre), A_(An), A_(tr1pre), Op.add)
                  rel(Ad, Bd, An, Bn)
                  trace = tmp()
                  V.tensor_scalar(A_(trace), A_(tr1pre), -1.0, None, Op.add)
                  V.tensor_scalar(m1[:, 0:F], A_(trace), float(EPS), None, Op.is_gt)
                  rel(trace)
                  # eps-clamps dropped: on the *selected* column the diagonal is
                  # >= ~2/3 for any orthonormal frame, so max(x, EPS) == x there;
                  # non-selected entries are never read.
                  tr1 = tr1pre
                  m00 = m00pre
                  m11 = m11pre
                  qz = tmp()
                  S.activation(A_(qz), A_(m22pre), AF.Copy)
                  rel(m22pre)
                  # combos
                  d1, d2, d3, p1, p2, p3 = tmp(), tmp(), tmp(), tmp(), tmp(), tmp()
                  TT("dp0", A_(d1), A_(b2z), A_(b3y), Op.subtract)
                  TT("dp1", A_(d2), A_(b3x), A_(b1z), Op.subtract)
                  TT("dp2", A_(d3), A_(b1y), A_(b2x), Op.subtract)
                  TT("dp3", A_(p1), A_(b2x), A_(b1y), Op.add)
                  TT("dp4", A_(p2), A_(b3x), A_(b1z), Op.add)
                  TT("dp5", A_(p3), A_(b3y), A_(b2z), Op.add)
                  rel(b1y, b1z, b2x, b2z, b3x, b3y, b3z)
                  # select: init with case4 (d3, p2, p3, m22->qz already)
                  qw, qx, qy = tmp(), tmp(), tmp()
                  S.activation(A_(qw), A_(d3), AF.Copy)
                  S.activation(A_(qx), A_(p2), AF.Copy)
                  S.activation(A_(qy), A_(p3), AF.Copy)
                  M1, M2, M3 = m1[:, 0:F], m2[:, 0:F], m3[:, 0:F]
                  if not no_pred:
                      V.copy_predicated(A_(qw), M3, A_(d2))
                      V.copy_predicated(A_(qx), M3, A_(p1))
                      V.copy_predicated(A_(qy), M3, A_(m11))
                      V.copy_predicated(A_(qz), M3, A_(p3))
                      V.copy_predicated(A_(qw), M2, A_(d1))
                      V.copy_predicated(A_(qx), M2, A_(m00))
                      V.copy_predicated(A_(qy), M2, A_(p1))
                      V.copy_predicated(A_(qz), M2, A_(p2))
                      V.copy_predicated(A_(qw), M1, A_(tr1))
                      V.copy_predicated(A_(qx), M1, A_(d1))
                      V.copy_predicated(A_(qy), M1, A_(d2))
                      V.copy_predicated(A_(qz), M1, A_(d3))
                  rel(tr1, m00, m11, d1, d2, d3, p1, p2, p3)
                  # nq & invq
                  u1, u2, u3 = tmp(), tmp(), tmp()
                  S.activation(A_(u1), A_(qw), AF.Square)
                  S.activation(A_(u2), A_(qx), AF.Square)
                  TT("nqa0", A_(u3), A_(u1), A_(u2), Op.add)
                  S.activation(A_(u1), A_(qy), AF.Square)
                  TT("nqa1", A_(u2), A_(u3), A_(u1), Op.add)
                  S.activation(A_(u1), A_(qz), AF.Square)
                  nq = u3
                  TT("nqa2", A_(nq), A_(u2), A_(u1), Op.add)
                  S.activation(A_(u1), A_(nq), AF.Ln)
                  invq = u2
                  S.activation(A_(invq), A_(u1), AF.Exp, scale=-0.5)
                  # final strided writes
                  TT("fin0", out_t[:, :, 6], A_(qw), A_(invq), Op.mult)
                  TT("fin1", out_t[:, :, 7], A_(qx), A_(invq), Op.mult)
                  TT("fin2", out_t[:, :, 8], A_(qy), A_(invq), Op.mult)
                  TT("fin3", out_t[:, :, 9], A_(qz), A_(invq), Op.mult)
                  rel(qw, qx, qy, qz, nq, u2, invq)

                  # serial scan cluster at chunk tail: its only cross-chunk
                  # deps (prev chunk's scans) are long satisfied by now, so it
                  # never stalls the in-order engine streams.
                  for c in range(3 if not no_scan else 0):
                      g_, e_, hv = scan_tiles[c]
                      init_v = v0t[:, c:c + 1] if ci == 0 else prev_out[:, F - 1:F, c]
                      SCAN_ENG.tensor_tensor_scan(
                          out_t[:, sl, c], g_[:, sl], g_[:, sl], init_v,
                          Op.add, Op.bypass)
                      if ci == 0:
                          V.tensor_copy(out_t[:, 0:1, c], v0t[:, c:c + 1])
                          V.tensor_copy(out_t[:, 0:1, 3 + c], z0t[:, c:c + 1])
                      # hv = dt * v(prev position)
                      TT(f"hv{c}", hv[:, 1:F], dt[:, 1:F], out_t[:, 0:F - 1, c],
                         Op.mult)
                      if ci > 0:
                          TT(f"hvh{c}", hv[:, 0:1], dt[:, 0:1],
                             prev_out[:, F - 1:F, c], Op.mult)
                      init_z = z0t[:, c:c + 1] if ci == 0 else prev_out[:, F - 1:F, 3 + c]
                      SCAN_ENG.tensor_tensor_scan(
                          out_t[:, sl, 3 + c], hv[:, sl], e_[:, sl], init_z,
                          Op.add, Op.add)
                  if not skip_dma:
                      nc.sync.dma_start(outd[:, n0:n0 + F, :], out_t[:])
                  prev_out = out_t
    nc.compile()
    return nc


# ----------------------------------------------------------------------------
# Public entry point: full inputs in, full output out (8-core data parallel).
# ----------------------------------------------------------------------------
from concourse import bass_utils as _bass_utils

_NC_CACHE = {}
N_CORES = 8
B_FULL = 1024
N_FULL = 4096
F_CHUNK = 512


def _get_nc():
    key = (B_FULL // N_CORES, N_FULL, F_CHUNK)
    if key not in _NC_CACHE:
        _NC_CACHE[key] = build_nc(BC=key[0], N=key[1], F=key[2])
    return _NC_CACHE[key]


def kernel(sixd, accel, v0, z0, t):
    sixd = np.ascontiguousarray(sixd, dtype=np.float32)
    accel = np.ascontiguousarray(accel, dtype=np.float32)
    v0 = np.ascontiguousarray(v0, dtype=np.float32)
    z0 = np.ascontiguousarray(z0, dtype=np.float32)
    t = np.ascontiguousarray(t, dtype=np.float32)
    B = sixd.shape[0]
    assert B == B_FULL, B
    BC = B // N_CORES
    nc = _get_nc()
    in_maps = []
    for i in range(N_CORES):
        sl = slice(i * BC, (i + 1) * BC)
        in_maps.append({
            "sixd": sixd[sl], "accel": accel[sl], "v0": v0[sl],
            "z0": z0[sl], "t": t[sl],
        })
    res = _bass_utils.run_bass_kernel_spmd(nc, in_maps,
                                           core_ids=list(range(N_CORES)))
    out = np.concatenate([r["out"] for r in res.results], axis=0)
    return out

